# revision 13
# baseline (speedup 1.0000x reference)
"""BigBird block-sparse attention on 8 Trainium2 NeuronCores — sparse exact-cover.

Problem: B=2, H=16, F=T=1024, D=64, 64x64-block BigBird mask per head
(generated at MAX_SEQ_LEN=4096 and cropped to 1024, so there is NO global
last row/col: row-block 15 attends only t-block 0).
  scores = (Q @ K^T) / 8 + (1-mask) * -10000
  out    = softmax(scores) @ V, laid out [B, F, H, D]

Sharding: head-parallel. Core c handles heads {2c, 2c+1} x both batches
= 4 (b,h) pairs; no cross-core communication.

Why sparse: the Activation engine (exp at ~1 col/cycle over 128 lanes)
is the bottleneck. The mask attends only 114 of 256 blocks per head:
  row 0: all 16            col 0: fb 1..15 (15)
  window {fb-1,fb,fb+1} for fb 1..14 (41 after dedup vs col 0)
  3 random blocks per fb 1..14 (42)
We compute S^T = K-slices^T x Q only on attended blocks, packed densely
into PSUM: exp processes exactly 3648 columns/pair (the 114-block floor)
instead of 8192. Skipped blocks contribute exactly 0 — identical to the
reference's exp(score - 1e4) -> 0 underflow in f32 — so no mask bias is
needed and the contraction stays 64.

The PROGRAM is identical on all cores: head-dependence (the random
blocks) is packed by the host into fixed slots of a gathered K region
(krem, appended to [q^T | K^T] in one DRAM tensor per pair) and gathered
V tiles (vhat, appended to the natural V tiles). Window/global pieces
use contiguous K slices and natural V tiles. t-blocks pair into
128-partition pieces; leftover 64-row singles from different from-blocks
share one chunk's two partition halves, so every PSUM column is fully
written (no uninit reads, no wasted exp columns).

S-tile layout per pair: [1536, 1024, 1088].
  tile 0: pieces needing only the dense K^T (a, sh, lcomb) — its QK can
          start as soon as the dense K/Q slices land, minimizing the head.
  tile 1: all remaining o_hi work (krem/vhat pieces, fb >= 8) — o_hi is
          complete after tile 1, so its normalize overlaps tile 2.
  tile 2: o_lo-only (fb < 8).  Its exp window (~0.9us) covers the next
          pair's tile-0 QK, removing the inter-pair ACT bubbles.

PV matmuls lag 3 stages behind the exp front (PE prioritizes QK, which
feeds the critical ACT engine); the lag adaptively shrinks to 0 over the
last stages so the final pair's PV does not pile up after the last exp.
The first exp of pair 0 and the last exp of the last pair are split at
piece boundaries to shorten the pipeline fill/drain.

Softmax denominator: V carries a ones-column (65th); the PV accumulation
yields rowsums for free; one DVE reciprocal + broadcast multiply
normalizes. No max-subtraction needed (scores ~ N(0,1), f32/f16 safe).

Output is stored in the PV-accumulator layout [128, 8, D] (contiguous
1KB DMA runs, no small-element penalty); the host un-permutes
(f = fo*128 + ti).

dtype: fp16 matmul inputs (f32 PSUM accumulate); output stored fp16 on
device (values O(1), fp16 rounding ~5e-4 rel), upcast to f32 on host.
"""

import numpy as np

B, H, F, T, D = 2, 16, 1024, 1024, 64
BS = 64                  # mask block size
NB = F // BS             # 16 blocks per axis
N_CORES = 8
HEADS_PER_CORE = H // N_CORES          # 2
PAIRS = HEADS_PER_CORE * B             # 4 (b,h) pairs per core

# --- static layout constants -------------------------------------------------
# One DRAM tensor per pair: [ q^T (1024) | K^T dense (1024) | krem (43*64) ]
QT_W = 1024
KT_OFF = QT_W
KT_W = 1024                    # dense K^T cols
KREM_OFF = KT_OFF + KT_W       # gathered K blocks appended after dense K^T
N_KREM = 56                    # 13*(r1,r2) + 13*[K0,r3] + [K15, r1(14)] + r2,r3(14)
QKT_W = KREM_OFF + N_KREM * BS           # 4800

VT_COLS = 8 * 65               # natural V tiles [t_in 128][tb 8][65]
VHAT_OFF = VT_COLS             # 520
N_VHAT = 31
VC_W = VHAT_OFF + N_VHAT * 65  # 2535

S_TILES = [1536, 1024, 1088]   # PSUM score-tile widths per pair (sum 3648)

_CACHE = {}


def _fb_dest(fb):
    """f-block -> (o_half, col_group, part0, part1) in the PV accumulators."""
    half = 0 if fb < 8 else 1
    grp = (fb % 8) // 2
    p0 = (fb % 2) * 64
    return half, grp, p0, p0 + 64


def _krem(slot, n=1):
    return KREM_OFF + BS * slot, KREM_OFF + BS * (slot + n)


def _kt(c0, c1):
    """Dense K^T cols -> qkt cols."""
    return KT_OFF + c0, KT_OFF + c1


def _vhat(slot):
    return VHAT_OFF + 65 * slot, VHAT_OFF + 65 * (slot + 1)


def _vt(tb, lower):
    """Natural V tile cols for t-block tb; lower half holds even blocks."""
    assert (tb % 2 == 0) == lower
    return 65 * (tb // 2), 65 * (tb // 2) + 65


def _build_plan():
    """Static piece list (identical for every head/core).

    piece: dict(tile, off, w, qk=[...], pv=[...])
      qk op: (p0, p1, kc0, kc1, q0, q1)        out parts p0:p1, free q1-q0
        kc* index the qkt tensor; q* are q cols (qkt cols 0..1024).
      pv op: (c0, c1, pp0, pp1, vc0, vc1, vp0, vp1, half, grp, op0, op1)
        c* cols relative to the piece's s-tile; v* index the v1c tensor.
    """
    pieces = []
    cur_tile, cur_off = 0, 0

    def add(w, qk, pv):
        nonlocal cur_tile, cur_off
        if cur_off + w > S_TILES[cur_tile]:
            assert cur_off == S_TILES[cur_tile], "packing hole"
            cur_tile += 1
            cur_off = 0
        # matmul dests must not cross a PSUM bank (512 f32 cols)
        assert (cur_off % 512) + w <= 512 or w % 512 == 0
        pieces.append(dict(tile=cur_tile, off=cur_off, w=w, qk=qk, pv=pv))
        cur_off += w

    def a_piece(j):
        # fb0 x t-tile j (natural tiles)
        add(64,
            [(0, 128) + _kt(128 * j, 128 * j + 128) + (0, 64)],
            [(0, 64, 0, 128, 65 * j, 65 * j + 65, 0, 128) + _fb_dest(0)])

    def sh(i):
        # t{2i,2i+1} x f{2i,2i+1} shared window piece (natural tiles)
        half, grp, _, _ = _fb_dest(2 * i)
        add(128,
            [(0, 128) + _kt(128 * i, 128 * i + 128) + (128 * i, 128 * i + 128)],
            [(0, 128, 0, 128, 65 * i, 65 * i + 65, 0, 128, half, grp, 0, 128)])

    def lcomb(i):
        # lower=(fb 2i+1, t 2i+2), upper=(fb 2i, t 2i-1); natural parity
        fe, fo = 2 * i, 2 * i + 1
        bl, bu = 2 * i + 2, 2 * i - 1
        add(64,
            [(0, 64) + _kt(64 * bl, 64 * bl + 64) + (64 * fo, 64 * fo + 64),
             (64, 128) + _kt(64 * bu, 64 * bu + 64) + (64 * fe, 64 * fe + 64)],
            [(0, 64, 0, 64) + _vt(bl, True) + (0, 64) + _fb_dest(fo),
             (0, 64, 64, 128) + _vt(bu, False) + (64, 128) + _fb_dest(fe)])

    def w1():
        add(64, [(0, 128) + _kt(64, 192) + (64, 128)],
            [(0, 64, 0, 128) + _vhat(0) + (0, 128) + _fb_dest(1)])

    def w14():
        add(64, [(0, 128) + _kt(832, 960) + (896, 960)],
            [(0, 64, 0, 128) + _vhat(1) + (0, 128) + _fb_dest(14)])
        kc0, kc1 = _krem(52, 2)
        add(64, [(0, 128, kc0, kc1, 896, 960)],
            [(0, 64, 0, 128) + _vhat(2) + (0, 128) + _fb_dest(14)])
        kc0, kc1 = _krem(54, 2)
        add(64, [(0, 128, kc0, kc1, 896, 960)],
            [(0, 64, 0, 128) + _vhat(16) + (0, 128) + _fb_dest(14)])

    def rp(fb):
        kc0, kc1 = _krem(2 * (fb - 1), 2)
        add(64, [(0, 128, kc0, kc1, 64 * fb, 64 * fb + 64)],
            [(0, 64, 0, 128) + _vhat(3 + (fb - 1)) + (0, 128) + _fb_dest(fb)])

    def single(fb):
        # chunk [t0 lower | r3(fb) upper]; both halves share the fb, so one
        # 128-contraction PV op against vhat [V0; Vr3] and ONE QK matmul
        # against the host-gathered [K0 | Kr3] krem pair suffice.
        kc0, kc1 = _krem(26 + 2 * (fb - 1), 2)
        vc0, vc1 = _vhat(17 + (fb - 1))
        add(64,
            [(0, 128, kc0, kc1, 64 * fb, 64 * fb + 64)],
            [(0, 64, 0, 128, vc0, vc1, 0, 128) + _fb_dest(fb)])

    def single_1415():
        # fb14/fb15 block-0 singles share one chunk; vhat30 = [V0; V0]
        vc0, vc1 = _vhat(30)
        add(64,
            [(0, 64) + _kt(0, 64) + (896, 960),
             (64, 128) + _kt(0, 64) + (960, 1024)],
            [(0, 64, 0, 64, vc0, vc1, 0, 64) + _fb_dest(14),
             (0, 64, 64, 128, vc0, vc1, 64, 128) + _fb_dest(15)])

    # tile 0 (1536): dense-K-only pieces, a-pieces first (earliest data)
    for j in range(8):
        a_piece(j)
    for i in range(1, 7):
        sh(i)
    for i in (4, 5, 6, 1):
        lcomb(i)
    # tile 1 (1024): all remaining o_hi work (krem/vhat pieces)
    w14()
    single_1415()
    for fb in range(8, 14):
        rp(fb)
    for fb in range(8, 14):
        single(fb)
    # tile 2 (1088): o_lo only
    for i in (2, 3):
        lcomb(i)
    for fb in range(1, 8):
        rp(fb)
    w1()
    for fb in range(1, 7):
        single(fb)
    single(7)          # offset 1024: the final 64-col exp chunk

    assert cur_tile == 2 and cur_off == S_TILES[2], (cur_tile, cur_off)
    # all o_hi PV must land by tile 1 (tile 2's PV writes only o_lo, so
    # the hi normalize at tile 1 creates no WAR hazard against them)
    hi_tiles = [pc["tile"] for pc in pieces for op in pc["pv"] if op[8] == 1]
    assert max(hi_tiles) == 1, hi_tiles

    # PV accumulation: the o tiles are memset-zeroed at pair start and every
    # PV matmul is a pure accumulate (start=False). Mixed 64/128-partition
    # accumulation brackets can't be expressed with PSUM zero-region start
    # flags (2KB region granularity x partition range), and a lazy
    # start_tensor_calc would re-mark already-written bytes pending-zero.
    tiles_qk = [[pc for pc in pieces if pc["tile"] == t] for t in range(3)]
    tiles_pv = [[(pc["off"], op) for pc in pieces if pc["tile"] == t
                 for op in pc["pv"]] for t in range(3)]

    # The o accumulators are zeroed by PSUM start_tensor_calc on the first
    # PV op per o-tile instead of DVE memsets: start=True marks the whole
    # 2KB zero region (the o tile's bank) pending-zero; each later op's
    # first touch of a byte replaces instead of accumulating.  The starter
    # must span partitions 0:128, so the full-height sh1 (o_lo) and sh4
    # (o_hi) ops are hoisted to the front of tile-0's PV list.  Every
    # byte of cols 0:260 is eventually written (the exact-cover invariant),
    # so no stale PSUM is ever read.
    pv0 = tiles_pv[0]

    def keyf(e):
        off, op = e
        half, p0, p1 = op[8], op[10], op[11]
        if half == 0 and p0 == 0 and p1 == 128 and off == 512:
            return 0                      # sh1 (o_lo starter)
        if half == 1 and p0 == 0 and p1 == 128 and off == 512 + 3 * 128:
            return 1                      # sh4 (o_hi starter)
        return 2
    pv0.sort(key=keyf)
    assert [keyf(e) for e in pv0[:2]] == [0, 1]
    starts = [[i < 2 for i in range(len(pv0))]] + [
        [False] * len(tiles_pv[t]) for t in (1, 2)]
    return tiles_qk, tiles_pv, starts


# Host packing maps. krem slot -> K source; vhat slot -> (lower64, upper64)
# sources. Source: ('d', tb) dense t-block | ('r', fb, i) i-th rand of fb.
def _krem_slot_map():
    slots = []
    for fb in range(1, 14):
        slots += [("r", fb, 0), ("r", fb, 1)]
    for fb in range(1, 14):
        slots += [("d", 0), ("r", fb, 2)]
    slots += [("d", 15), ("r", 14, 0), ("r", 14, 1), ("r", 14, 2)]
    assert len(slots) == N_KREM
    return slots


def _vhat_slot_map():
    slots = [(("d", 1), ("d", 2)), (("d", 13), ("d", 14)),
             (("d", 15), ("r", 14, 0))]
    slots += [(("r", fb, 0), ("r", fb, 1)) for fb in range(1, 14)]
    slots += [(("r", 14, 1), ("r", 14, 2))]
    slots += [(("d", 0), ("r", fb, 2)) for fb in range(1, 14)]
    slots += [(("d", 0), ("d", 0))]
    assert len(slots) == N_VHAT
    return slots


def _head_rand_blocks(bm):
    """Per-from-block rand lists from a [16,16] block mask, validating the
    cropped-BigBird structure this kernel's decomposition assumes."""
    if not bm[0].all():
        raise ValueError("row-block 0 not global")
    if not bm[:, 0].all():
        raise ValueError("col-block 0 not global")
    rand = {}
    for fb in range(1, 15):
        win = {fb - 1, fb, fb + 1} & set(range(16))
        att = {tb for tb in range(16) if bm[fb, tb]}
        if not win <= att:
            raise ValueError(f"window blocks missing for fb={fb}")
        r = sorted(att - win - {0})
        if len(r) > 3:
            raise ValueError(f"more than 3 rand blocks for fb={fb}")
        rand[fb] = r
    if not np.array_equal(bm[15], np.eye(16, dtype=bool)[0]):
        raise ValueError("row-block 15 must attend exactly t-block 0")
    # coverage: pieces must cover the support exactly once
    cov = np.zeros((16, 16), dtype=np.int32)
    cov[0, :] += 1
    cov[1:16, 0] += 1
    for fb in range(1, 15):
        for tb in ({fb - 1, fb, fb + 1} & set(range(16))) - {0}:
            cov[fb, tb] += 1
        for tb in rand[fb]:
            cov[fb, tb] += 1
    if not np.array_equal(cov, bm.astype(np.int32)):
        raise ValueError("decomposition does not cover the mask exactly")
    return rand


def _exp_dve_ops():
    """Register (once) the two custom DVE ops that evaluate exp on the
    Vector engine: exp(s*x) = core(x)^128 with core = 1 + t + t^2/2,
    t = s*x/128 (s = 0.125 softmax scale).  Log-domain error s^3*x^3/
    (6*128^2) ~ 1.3e-3 at |s*x| = 5 — comparable to fp16 rounding.
    Registration follows the documented per-NEFF DVE-table path
    (bass_utils.dve_table_for_ops resolves names via dve_ops.OPS).
    """
    if "dve_ops" in _CACHE:
        return _CACHE["dve_ops"]
    import numpy as np
    from concourse.dve_spec import Spec, Src0, C0, C1, One, sq, lower
    from concourse.dve_ops import (
        DveOp, OPS, CUSTOM_DVE_SPECS, _SUB_OPCODE_FOR_NAME)
    from concourse.dve_uop import DveOpSpec

    def register(name, spec, rd1):
        if name in _SUB_OPCODE_FOR_NAME:
            return next(op for op in OPS if op.name == name)
        row = max(_SUB_OPCODE_FOR_NAME.values()) + 1
        assert row < 0x20
        shas = {}
        for ver in ("v3", "v4"):
            s = DveOpSpec(name=name, opcode=row, uops=lower(spec, ver=ver),
                          rd1_en=rd1)
            shas[ver] = s.sha(ver)
        op = DveOp(name, spec, subdim=False, uops_sha=shas)
        _SUB_OPCODE_FOR_NAME[name] = row
        CUSTOM_DVE_SPECS[name] = spec
        OPS.append(op)
        return op

    def core_ref(in0, in1, c0, c1, c2):
        x = np.asarray(in0, np.float32)
        t = x * np.float32(c0)
        m = x * np.float32(c1)
        return (np.float32(1.0) + t) + m * m

    def sq7_ref(in0, in1, c0, c1, c2):
        p = np.asarray(in0, np.float32)
        for _ in range(7):
            p = p * p
        return p

    core = register(
        "EXP_CORE_D2_ANT",
        Spec(body=(One + Src0 * C0) + sq(Src0 * C1), reference=core_ref),
        rd1=False)
    x = Src0
    for _ in range(7):
        x = sq(x)
    sq7 = register("EXP_SQ7_ANT", Spec(body=x, reference=sq7_ref), rd1=False)
    _CACHE["dve_ops"] = (core, sq7)
    return core, sq7


# Per-(pair, tile) exp chunking: [c0, c1, engine].  "D" chunks run on the
# Vector engine (2 custom ops via an f32 scratch), offloading ~1/6 of the
# exp columns from the critical Activation engine.  Pair 3 offloads its
# a-piece chunk (tile 0) instead of tile 2 so the DVE never gates the tail;
# pair 0 keeps tile 0 on ACT (head-critical) with a 3-way split so the
# first exp starts after only 4 a-piece matmuls.
_EXP_CHUNKS = {
    (0, 0): [(0, 256, "A"), (256, 512, "A"), (512, 1536, "A")],
    (0, 1): [(0, 512, "D"), (512, 1024, "A")],
    (1, 1): [(0, 512, "D"), (512, 1024, "A")],
    (2, 1): [(0, 512, "D"), (512, 1024, "A")],
    (3, 1): [(0, 512, "D"), (512, 1024, "A")],
    (0, 2): [(0, 512, "D"), (512, 1088, "A")],
    (1, 2): [(0, 512, "D"), (512, 1088, "A")],
    (2, 2): [(0, 512, "D"), (512, 1088, "A")],
    (3, 0): [(0, 512, "D"), (512, 1536, "A")],
    (3, 2): [(0, 512, "A"), (512, 1024, "A"), (1024, 1088, "A")],
}

_EXP_S = 0.125                  # softmax 1/sqrt(d)
_EXP_N = 128.0                  # squaring ladder height (2^7)


def _build_nc():
    """Build + finalize the per-core Bass program (identical on all cores)."""
    import concourse.tile as tile
    from concourse import bacc, mybir

    tiles_qk, tiles_pv, pv_starts = _build_plan()
    exp_core, exp_sq7 = _exp_dve_ops()

    nc = bacc.Bacc(None, target_bir_lowering=False)
    f16 = mybir.dt.float16
    f32 = mybir.dt.float32

    qkt = nc.dram_tensor("qkt", [PAIRS, 64, QKT_W], f16, kind="ExternalInput")
    v1 = nc.dram_tensor("v1", [PAIRS, 128, VC_W], f16, kind="ExternalInput")
    out = nc.dram_tensor("out", [PAIRS, 128, 8, D], f16, kind="ExternalOutput")

    Exp = mybir.ActivationFunctionType.Exp

    with tile.TileContext(nc) as tc:
        with (
            tc.tile_pool(name="io", bufs=4) as io_pool,
            tc.tile_pool(name="pt", bufs=4) as pt_pool,
            tc.tile_pool(name="res", bufs=6) as res_pool,
            tc.tile_pool(name="esc", bufs=2) as esc_pool,
            tc.tile_pool(name="spsum", bufs=2, space="PSUM") as s_psum,
            tc.tile_pool(name="opsum", bufs=2, space="PSUM") as o_psum,
        ):
            state = {}

            # ACT table preload: a 1-col exp on a memset scratch makes the
            # 1.28us activation-table load happen during the DMA head phase
            # instead of on the first real exp.
            scr = res_pool.tile([128, 2], f32, tag="scr")
            scr16 = res_pool.tile([128, 2], f16, tag="scr16")
            nc.vector.memset(scr[:], 0.0)
            nc.scalar.activation(scr16[:, 0:1], scr[:, 0:1], Exp)

            def emit_load(p):
                kq = io_pool.tile([64, QKT_W], f16, tag="kq")
                vc = io_pool.tile([128, VC_W], f16, tag="vc")
                # qkt on the gpsimd queue, vc on sync: transfers parallelize
                # across queues. Pair 0 splits off the tiny slices its first
                # matmuls need (the first-arriving transfer should be minimal)
                if p == 0:
                    nc.gpsimd.dma_start(out=kq[:, KT_OFF:KT_OFF + 128],
                                        in_=qkt[p, :, KT_OFF:KT_OFF + 128])
                    nc.sync.dma_start(out=kq[:, 0:64], in_=qkt[p, :, 0:64])
                    nc.gpsimd.dma_start(out=kq[:, KT_OFF + 128:KREM_OFF],
                                        in_=qkt[p, :, KT_OFF + 128:KREM_OFF])
                    nc.sync.dma_start(out=kq[:, 64:KT_OFF],
                                      in_=qkt[p, :, 64:KT_OFF])
                    nc.gpsimd.dma_start(out=kq[:, KREM_OFF:],
                                        in_=qkt[p, :, KREM_OFF:])
                elif p in (1, 2):
                    # pairs 1-2: q + dense K ride the sync queue (the gpsimd
                    # queue is busy with the krem streams); krem on gpsimd
                    nc.sync.dma_start(out=kq[:, 0:KREM_OFF],
                                      in_=qkt[p, :, 0:KREM_OFF])
                    nc.gpsimd.dma_start(out=kq[:, KREM_OFF:],
                                        in_=qkt[p, :, KREM_OFF:])
                else:
                    nc.sync.dma_start(out=kq[:, 0:KT_OFF],
                                      in_=qkt[p, :, 0:KT_OFF])
                    nc.gpsimd.dma_start(out=kq[:, KT_OFF:],
                                        in_=qkt[p, :, KT_OFF:])
                nc.sync.dma_start(out=vc[:], in_=v1[p, :, :])
                state[p] = dict(kq=kq, vc=vc, pts=[])

            def emit_front(p, ti):
                st = state[p]
                if ti == 0:
                    # width 512 (not 260): exactly one 2KB PSUM bank per
                    # partition, so the start_tensor_calc zero-region (2KB
                    # granular) aligns exactly with the tile — no marking
                    # bleed into neighbours, no OOB on the last partition
                    o_lo = o_psum.tile([128, 512], f32, tag="o")
                    o_hi = o_psum.tile([128, 512], f32, tag="o")
                    st["o"] = (o_lo, o_hi)
                w = S_TILES[ti]
                s_ps = s_psum.tile([128, w], f32, tag="s")
                kq = st["kq"]
                for pc in tiles_qk[ti]:
                    off = pc["off"]
                    for (p0, p1, kc0, kc1, q0, q1) in pc["qk"]:
                        nc.tensor.matmul(
                            s_ps[p0:p1, off:off + (q1 - q0)],
                            lhsT=kq[:, kc0:kc1], rhs=kq[:, q0:q1],
                            start=True, stop=True,
                        )
                pt = pt_pool.tile([128, w], f16, tag="p")
                st["pts"].append(pt)
                dve_chunks = []
                for (c0, c1, eng) in _EXP_CHUNKS.get((p, ti), [(0, w, "A")]):
                    if eng == "A":
                        nc.scalar.activation(pt[:, c0:c1], s_ps[:, c0:c1],
                                             Exp, scale=_EXP_S)
                    else:
                        dve_chunks.append((s_ps, pt, c0, c1))
                return dve_chunks

            def emit_pv(p, ti):
                st = state[p]
                pt = st["pts"][ti]
                vc = st["vc"]
                for (off, (c0, c1, pp0, pp1, vc0, vc1, vp0, vp1,
                           half, grp, op0, op1)), st_flag in zip(
                               tiles_pv[ti], pv_starts[ti]):
                    o_ps = st["o"][half]
                    nc.tensor.matmul(
                        o_ps[op0:op1, grp * 65:(grp + 1) * 65],
                        lhsT=pt[pp0:pp1, off + c0:off + c1],
                        rhs=vc[vp0:vp1, vc0:vc1],
                        start=st_flag,
                        stop=False,
                        skip_group_check=True,
                    )
                if ti == 1:
                    # o_hi complete after tile 1; normalize it here so the
                    # work overlaps tile 2. Only the last pair stores hi
                    # immediately (other pairs merge into one store below).
                    emit_norm(p, 1)
                    if p == PAIRS - 1:
                        nc.sync.dma_start(out=out[p, :, 4:8, :],
                                          in_=state[p]["os"][:, 4:8, :])
                elif ti == 2:
                    emit_norm(p, 0)
                    os = st["os"]
                    if p == PAIRS - 1:
                        nc.gpsimd.dma_start(out=out[p, :, 0:2, :],
                                            in_=os[:, 0:2, :])
                        nc.sync.dma_start(out=out[p, :, 2:4, :],
                                          in_=os[:, 2:4, :])
                    elif p == 2:
                        nc.sync.dma_start(out=out[p], in_=os[:])
                    else:
                        nc.gpsimd.dma_start(out=out[p], in_=os[:])
                    del state[p]

            def emit_norm(p, half):
                st = state[p]
                if "os" not in st:
                    o_sb = res_pool.tile([128, 8, D], f16, tag="os")
                    recip = res_pool.tile([128, 8], f32, tag="r")
                    st["os"], st["rc"] = o_sb, recip
                o_ps = st["o"][half]
                rc_all = st["rc"][:, half * 4: half * 4 + 4]
                nc.vector.reciprocal(rc_all, o_ps[:, 64:260:65])
                if p == PAIRS - 1 and half == 0:
                    chains = [(0, 2), (2, 4)]
                else:
                    chains = [(0, 4)]
                for (g0, g1) in chains:
                    f0, f1 = half * 4 + g0, half * 4 + g1
                    nc.vector.tensor_mul(
                        st["os"][:, f0:f1, :],
                        o_ps[:, 65 * g0:65 * g1].rearrange(
                            "p (fi c) -> p fi c", c=65)[:, :, 0:64],
                        st["rc"][:, f0:f1].to_broadcast([128, g1 - g0, 64]),
                    )

            for p in range(PAIRS):
                emit_load(p)
            stages = [(p, ti) for p in range(PAIRS) for ti in range(3)]
            n = len(stages)
            pv_next = 0
            def emit_dve_exp(s_ps, pt, c0, c1):
                scr = esc_pool.tile([128, 512], f32, tag="e")
                nc.vector._custom_dve(
                    exp_core, out=scr[:, 0:c1 - c0], in0=s_ps[:, c0:c1],
                    s0=_EXP_S / _EXP_N, s1=_EXP_S / (_EXP_N * 2.0 ** 0.5))
                nc.vector._custom_dve(
                    exp_sq7, out=pt[:, c0:c1], in0=scr[:, 0:c1 - c0])

            for s, (p, ti) in enumerate(stages):
                dve_chunks = emit_front(p, ti)
                # PE prioritizes QK (feeds ACT); drain the lag at the end so
                # the last pair's PV does not pile up after the final exp.
                # Keep lag >= 2 until the last front so pv(3,0) (gated on
                # exp(3,0)) is emitted after QK(3,2), not before it.
                lag = 3 if s < n - 3 else (2 if s < n - 1 else 0)
                while pv_next <= s - lag:
                    emit_pv(*stages[pv_next])
                    pv_next += 1
                # DVE exp chunks go behind this stage's norm muls in the
                # DVE FIFO (they have more slack than the store chain)
                for ch in dve_chunks:
                    emit_dve_exp(*ch)
            while pv_next < n:
                emit_pv(*stages[pv_next])
                pv_next += 1

    nc.finalize()
    return nc


def _prep_inputs(query_layer, key_layer, value_layer, attention_mask):
    """Host-side shard prep: per-core input maps."""
    bf = np.float16
    q = np.asarray(query_layer, dtype=np.float32)
    k = np.asarray(key_layer, dtype=np.float32)
    v = np.asarray(value_layer, dtype=np.float32)
    m = np.asarray(attention_mask, dtype=np.float32)

    # mask must be constant within 64x64 blocks (checked on a few offsets)
    mrow = m[0, :, ::BS, :]                      # [H, NB, T]
    for off in (17, 63):
        if not np.array_equal(mrow, m[0, :, off::BS, :]):
            raise ValueError("mask not constant within 64-row blocks")
    bms = mrow[:, :, ::BS]                       # [H, NB, NB]
    for off in (17, 63):
        if not np.array_equal(bms, mrow[:, :, off::BS]):
            raise ValueError("mask not constant within 64-col blocks")
    bms = bms.astype(bool)

    heads = {h: _head_rand_blocks(bms[h]) for h in range(H)}
    kslots = _krem_slot_map()
    vslots = _vhat_slot_map()

    qT = q.transpose(0, 1, 3, 2).astype(bf)             # [B, H, D, F]
    kT = k.transpose(0, 1, 3, 2).astype(bf)             # [B, H, D, T]
    v1f = np.concatenate(
        [v.astype(bf), np.ones((B, H, T, 1), dtype=bf)], axis=-1
    )                                                   # [B, H, T, 65]
    v1r = v1f.reshape(B, H, 8, 128, 65).transpose(0, 1, 3, 2, 4).reshape(
        B, H, 128, 8 * 65)

    in_maps = []
    pair_index = []
    for c in range(N_CORES):
        qkt = np.zeros((PAIRS, 64, QKT_W), dtype=bf)
        v1c = np.zeros((PAIRS, 128, VC_W), dtype=bf)
        pairs = []
        for p in range(PAIRS):
            h = HEADS_PER_CORE * c + p // B
            b = p % B
            rand = heads[h]

            def blk(src):
                if src[0] == "d":
                    return src[1]
                _, fb, i = src
                r = rand[fb]
                return r[i] if i < len(r) else None

            qkt[p, :, 0:QT_W] = qT[b, h]
            qkt[p, :, KT_OFF:KREM_OFF] = kT[b, h]
            for si, src in enumerate(kslots):
                tb = blk(src)
                if tb is not None:
                    c0 = KREM_OFF + BS * si
                    qkt[p, :, c0:c0 + 64] = kT[b, h][:, BS * tb:BS * tb + 64]
            v1c[p, :, 0:VT_COLS] = v1r[b, h]
            for si, (lo, hi) in enumerate(vslots):
                c0 = VHAT_OFF + 65 * si
                bl, bu = blk(lo), blk(hi)
                if bl is not None:
                    v1c[p, 0:64, c0:c0 + 65] = v1f[b, h, BS * bl:BS * bl + 64]
                if bu is not None:
                    v1c[p, 64:128, c0:c0 + 65] = v1f[b, h, BS * bu:BS * bu + 64]
            pairs.append((b, h))
        in_maps.append({"qkt": qkt, "v1": v1c})
        pair_index.append(pairs)
    return in_maps, pair_index


def kernel(query_layer, key_layer, value_layer, attention_mask):
    from concourse.bass_utils import run_bass_kernel_spmd

    if "nc" not in _CACHE:
        _CACHE["nc"] = _build_nc()
    nc = _CACHE["nc"]

    in_maps, pair_index = _prep_inputs(
        query_layer, key_layer, value_layer, attention_mask
    )
    core_ids = list(range(N_CORES))
    try:
        res = run_bass_kernel_spmd(nc, in_maps, core_ids)
    except Exception:
        # transient device errors clear on redispatch
        res = run_bass_kernel_spmd(nc, in_maps, core_ids)

    out = np.empty((B, F, H, D), dtype=np.float32)
    for c in range(N_CORES):
        core_out = res.results[c]["out"]         # [PAIRS, 128, 8, D]
        for p, (b, h) in enumerate(pair_index[c]):
            # un-permute: f = fo*128 + ti
            out[b, :, h, :] = core_out[p].transpose(1, 0, 2).reshape(F, D)
    return out


# revision 14
# speedup vs baseline: 1.0288x; 1.0288x over previous
"""BigBird block-sparse attention on 8 Trainium2 NeuronCores — sparse exact-cover.

Problem: B=2, H=16, F=T=1024, D=64, 64x64-block BigBird mask per head
(generated at MAX_SEQ_LEN=4096 and cropped to 1024, so there is NO global
last row/col: row-block 15 attends only t-block 0).
  scores = (Q @ K^T) / 8 + (1-mask) * -10000
  out    = softmax(scores) @ V, laid out [B, F, H, D]

Sharding: head-parallel. Core c handles heads {2c, 2c+1} x both batches
= 4 (b,h) pairs; no cross-core communication.

Why sparse: the Activation engine (exp at ~1 col/cycle over 128 lanes)
is the bottleneck. The mask attends only 114 of 256 blocks per head:
  row 0: all 16            col 0: fb 1..15 (15)
  window {fb-1,fb,fb+1} for fb 1..14 (41 after dedup vs col 0)
  3 random blocks per fb 1..14 (42)
We compute S^T = K-slices^T x Q only on attended blocks, packed densely
into PSUM: exp processes exactly 3648 columns/pair (the 114-block floor)
instead of 8192. Skipped blocks contribute exactly 0 — identical to the
reference's exp(score - 1e4) -> 0 underflow in f32 — so no mask bias is
needed and the contraction stays 64.

The PROGRAM is identical on all cores: head-dependence (the random
blocks) is packed by the host into fixed slots of a gathered K region
(krem, appended to [q^T | K^T] in one DRAM tensor per pair) and gathered
V tiles (vhat, appended to the natural V tiles). Window/global pieces
use contiguous K slices and natural V tiles. t-blocks pair into
128-partition pieces; leftover 64-row singles from different from-blocks
share one chunk's two partition halves, so every PSUM column is fully
written (no uninit reads, no wasted exp columns).

S-tile layout per pair: [1536, 1024, 1088].
  tile 0: pieces needing only the dense K^T (a, sh, lcomb) — its QK can
          start as soon as the dense K/Q slices land, minimizing the head.
  tile 1: all remaining o_hi work (krem/vhat pieces, fb >= 8) — o_hi is
          complete after tile 1, so its normalize overlaps tile 2.
  tile 2: o_lo-only (fb < 8).  Its exp window (~0.9us) covers the next
          pair's tile-0 QK, removing the inter-pair ACT bubbles.

PV matmuls lag 3 stages behind the exp front (PE prioritizes QK, which
feeds the critical ACT engine); the lag adaptively shrinks to 0 over the
last stages so the final pair's PV does not pile up after the last exp.
The first exp of pair 0 and the last exp of the last pair are split at
piece boundaries to shorten the pipeline fill/drain.

Softmax denominator: V carries a ones-column (65th); the PV accumulation
yields rowsums for free; one DVE reciprocal + broadcast multiply
normalizes. No max-subtraction needed (scores ~ N(0,1), f32/f16 safe).

Output is stored in the PV-accumulator layout [128, 8, D] (contiguous
1KB DMA runs, no small-element penalty); the host un-permutes
(f = fo*128 + ti).

dtype: fp16 matmul inputs (f32 PSUM accumulate); output stored fp16 on
device (values O(1), fp16 rounding ~5e-4 rel), upcast to f32 on host.
"""

import numpy as np

B, H, F, T, D = 2, 16, 1024, 1024, 64
BS = 64                  # mask block size
NB = F // BS             # 16 blocks per axis
N_CORES = 8
HEADS_PER_CORE = H // N_CORES          # 2
PAIRS = HEADS_PER_CORE * B             # 4 (b,h) pairs per core

# --- static layout constants -------------------------------------------------
# One DRAM tensor per pair: [ q^T (1024) | K^T dense (1024) | krem (43*64) ]
QT_W = 1024
KT_OFF = QT_W
KT_W = 1024                    # dense K^T cols
KREM_OFF = KT_OFF + KT_W       # gathered K blocks appended after dense K^T
N_KREM = 56                    # 13*(r1,r2) + 13*[K0,r3] + [K15, r1(14)] + r2,r3(14)
QKT_W = KREM_OFF + N_KREM * BS           # 4800

VT_COLS = 8 * 65               # natural V tiles [t_in 128][tb 8][65]
VHAT_OFF = VT_COLS             # 520
N_VHAT = 31
VC_W = VHAT_OFF + N_VHAT * 65  # 2535

S_TILES = [1536, 1024, 1088]   # PSUM score-tile widths per pair (sum 3648)

_CACHE = {}


def _fb_dest(fb):
    """f-block -> (o_half, col_group, part0, part1) in the PV accumulators."""
    half = 0 if fb < 8 else 1
    grp = (fb % 8) // 2
    p0 = (fb % 2) * 64
    return half, grp, p0, p0 + 64


def _krem(slot, n=1):
    return KREM_OFF + BS * slot, KREM_OFF + BS * (slot + n)


def _kt(c0, c1):
    """Dense K^T cols -> qkt cols."""
    return KT_OFF + c0, KT_OFF + c1


def _vhat(slot):
    return VHAT_OFF + 65 * slot, VHAT_OFF + 65 * (slot + 1)


def _vt(tb, lower):
    """Natural V tile cols for t-block tb; lower half holds even blocks."""
    assert (tb % 2 == 0) == lower
    return 65 * (tb // 2), 65 * (tb // 2) + 65


def _build_plan():
    """Static piece list (identical for every head/core).

    piece: dict(tile, off, w, qk=[...], pv=[...])
      qk op: (p0, p1, kc0, kc1, q0, q1)        out parts p0:p1, free q1-q0
        kc* index the qkt tensor; q* are q cols (qkt cols 0..1024).
      pv op: (c0, c1, pp0, pp1, vc0, vc1, vp0, vp1, half, grp, op0, op1)
        c* cols relative to the piece's s-tile; v* index the v1c tensor.
    """
    pieces = []
    cur_tile, cur_off = 0, 0

    def add(w, qk, pv):
        nonlocal cur_tile, cur_off
        if cur_off + w > S_TILES[cur_tile]:
            assert cur_off == S_TILES[cur_tile], "packing hole"
            cur_tile += 1
            cur_off = 0
        # matmul dests must not cross a PSUM bank (512 f32 cols)
        assert (cur_off % 512) + w <= 512 or w % 512 == 0
        pieces.append(dict(tile=cur_tile, off=cur_off, w=w, qk=qk, pv=pv))
        cur_off += w

    def a_piece(j):
        # fb0 x t-tile j (natural tiles)
        add(64,
            [(0, 128) + _kt(128 * j, 128 * j + 128) + (0, 64)],
            [(0, 64, 0, 128, 65 * j, 65 * j + 65, 0, 128) + _fb_dest(0)])

    def sh(i):
        # t{2i,2i+1} x f{2i,2i+1} shared window piece (natural tiles)
        half, grp, _, _ = _fb_dest(2 * i)
        add(128,
            [(0, 128) + _kt(128 * i, 128 * i + 128) + (128 * i, 128 * i + 128)],
            [(0, 128, 0, 128, 65 * i, 65 * i + 65, 0, 128, half, grp, 0, 128)])

    def lcomb(i):
        # lower=(fb 2i+1, t 2i+2), upper=(fb 2i, t 2i-1); natural parity
        fe, fo = 2 * i, 2 * i + 1
        bl, bu = 2 * i + 2, 2 * i - 1
        add(64,
            [(0, 64) + _kt(64 * bl, 64 * bl + 64) + (64 * fo, 64 * fo + 64),
             (64, 128) + _kt(64 * bu, 64 * bu + 64) + (64 * fe, 64 * fe + 64)],
            [(0, 64, 0, 64) + _vt(bl, True) + (0, 64) + _fb_dest(fo),
             (0, 64, 64, 128) + _vt(bu, False) + (64, 128) + _fb_dest(fe)])

    def w1():
        add(64, [(0, 128) + _kt(64, 192) + (64, 128)],
            [(0, 64, 0, 128) + _vhat(0) + (0, 128) + _fb_dest(1)])

    def w14():
        add(64, [(0, 128) + _kt(832, 960) + (896, 960)],
            [(0, 64, 0, 128) + _vhat(1) + (0, 128) + _fb_dest(14)])
        kc0, kc1 = _krem(24, 2)
        add(64, [(0, 128, kc0, kc1, 896, 960)],
            [(0, 64, 0, 128) + _vhat(2) + (0, 128) + _fb_dest(14)])
        kc0, kc1 = _krem(26, 2)
        add(64, [(0, 128, kc0, kc1, 896, 960)],
            [(0, 64, 0, 128) + _vhat(16) + (0, 128) + _fb_dest(14)])

    def rp(fb):
        kc0, kc1 = _krem(_krem_rp_slot(fb), 2)
        add(64, [(0, 128, kc0, kc1, 64 * fb, 64 * fb + 64)],
            [(0, 64, 0, 128) + _vhat(3 + (fb - 1)) + (0, 128) + _fb_dest(fb)])

    def single(fb):
        # chunk [t0 lower | r3(fb) upper]; both halves share the fb, so one
        # 128-contraction PV op against vhat [V0; Vr3] and ONE QK matmul
        # against the host-gathered [K0 | Kr3] krem pair suffice.
        kc0, kc1 = _krem(_krem_single_slot(fb), 2)
        vc0, vc1 = _vhat(17 + (fb - 1))
        add(64,
            [(0, 128, kc0, kc1, 64 * fb, 64 * fb + 64)],
            [(0, 64, 0, 128, vc0, vc1, 0, 128) + _fb_dest(fb)])

    def single_1415():
        # fb14/fb15 block-0 singles share one chunk; vhat30 = [V0; V0]
        vc0, vc1 = _vhat(30)
        add(64,
            [(0, 64) + _kt(0, 64) + (896, 960),
             (64, 128) + _kt(0, 64) + (960, 1024)],
            [(0, 64, 0, 64, vc0, vc1, 0, 64) + _fb_dest(14),
             (0, 64, 64, 128, vc0, vc1, 64, 128) + _fb_dest(15)])

    # tile 0 (1536): dense-K-only pieces, a-pieces first (earliest data)
    for j in range(8):
        a_piece(j)
    for i in range(1, 7):
        sh(i)
    for i in (4, 5, 6, 1):
        lcomb(i)
    # tile 1 (1024): all remaining o_hi work (krem/vhat pieces)
    w14()
    single_1415()
    for fb in range(8, 14):
        rp(fb)
    for fb in range(8, 14):
        single(fb)
    # tile 2 (1088): o_lo only
    for i in (2, 3):
        lcomb(i)
    for fb in range(1, 8):
        rp(fb)
    w1()
    for fb in range(1, 7):
        single(fb)
    single(7)          # offset 1024: the final 64-col exp chunk

    assert cur_tile == 2 and cur_off == S_TILES[2], (cur_tile, cur_off)
    # all o_hi PV must land by tile 1 (tile 2's PV writes only o_lo, so
    # the hi normalize at tile 1 creates no WAR hazard against them)
    hi_tiles = [pc["tile"] for pc in pieces for op in pc["pv"] if op[8] == 1]
    assert max(hi_tiles) == 1, hi_tiles

    # PV accumulation: the o tiles are memset-zeroed at pair start and every
    # PV matmul is a pure accumulate (start=False). Mixed 64/128-partition
    # accumulation brackets can't be expressed with PSUM zero-region start
    # flags (2KB region granularity x partition range), and a lazy
    # start_tensor_calc would re-mark already-written bytes pending-zero.
    tiles_qk = [[pc for pc in pieces if pc["tile"] == t] for t in range(3)]
    tiles_pv = [[(pc["off"], op) for pc in pieces if pc["tile"] == t
                 for op in pc["pv"]] for t in range(3)]

    # The o accumulators are zeroed by PSUM start_tensor_calc on the first
    # PV op per o-tile instead of DVE memsets: start=True marks the whole
    # 2KB zero region (the o tile's bank) pending-zero; each later op's
    # first touch of a byte replaces instead of accumulating.  The starter
    # must span partitions 0:128, so the full-height sh1 (o_lo) and sh4
    # (o_hi) ops are hoisted to the front of tile-0's PV list.  Every
    # byte of cols 0:260 is eventually written (the exact-cover invariant),
    # so no stale PSUM is ever read.
    pv0 = tiles_pv[0]

    def keyf(e):
        off, op = e
        half, p0, p1 = op[8], op[10], op[11]
        if half == 0 and p0 == 0 and p1 == 128 and off == 512:
            return 0                      # sh1 (o_lo starter)
        if half == 1 and p0 == 0 and p1 == 128 and off == 512 + 3 * 128:
            return 1                      # sh4 (o_hi starter)
        return 2
    pv0.sort(key=keyf)
    assert [keyf(e) for e in pv0[:2]] == [0, 1]
    starts = [[i < 2 for i in range(len(pv0))]] + [
        [False] * len(tiles_pv[t]) for t in (1, 2)]
    return tiles_qk, tiles_pv, starts


# Host packing maps. krem slot -> K source; vhat slot -> (lower64, upper64)
# sources. Source: ('d', tb) dense t-block | ('r', fb, i) i-th rand of fb.
KREM_SPLIT = 28                # tile-1 uses slots [0:28), tile-2 [28:56)


def _krem_rp_slot(fb):
    return 2 * (fb - 8) if fb >= 8 else KREM_SPLIT + 2 * (fb - 1)


def _krem_single_slot(fb):
    return 12 + 2 * (fb - 8) if fb >= 8 else KREM_SPLIT + 14 + 2 * (fb - 1)


def _krem_slot_map():
    slots = []
    for fb in range(8, 14):
        slots += [("r", fb, 0), ("r", fb, 1)]
    for fb in range(8, 14):
        slots += [("d", 0), ("r", fb, 2)]
    slots += [("d", 15), ("r", 14, 0), ("r", 14, 1), ("r", 14, 2)]
    for fb in range(1, 8):
        slots += [("r", fb, 0), ("r", fb, 1)]
    for fb in range(1, 8):
        slots += [("d", 0), ("r", fb, 2)]
    assert len(slots) == N_KREM
    return slots


def _vhat_slot_map():
    slots = [(("d", 1), ("d", 2)), (("d", 13), ("d", 14)),
             (("d", 15), ("r", 14, 0))]
    slots += [(("r", fb, 0), ("r", fb, 1)) for fb in range(1, 14)]
    slots += [(("r", 14, 1), ("r", 14, 2))]
    slots += [(("d", 0), ("r", fb, 2)) for fb in range(1, 14)]
    slots += [(("d", 0), ("d", 0))]
    assert len(slots) == N_VHAT
    return slots


def _head_rand_blocks(bm):
    """Per-from-block rand lists from a [16,16] block mask, validating the
    cropped-BigBird structure this kernel's decomposition assumes."""
    if not bm[0].all():
        raise ValueError("row-block 0 not global")
    if not bm[:, 0].all():
        raise ValueError("col-block 0 not global")
    rand = {}
    for fb in range(1, 15):
        win = {fb - 1, fb, fb + 1} & set(range(16))
        att = {tb for tb in range(16) if bm[fb, tb]}
        if not win <= att:
            raise ValueError(f"window blocks missing for fb={fb}")
        r = sorted(att - win - {0})
        if len(r) > 3:
            raise ValueError(f"more than 3 rand blocks for fb={fb}")
        rand[fb] = r
    if not np.array_equal(bm[15], np.eye(16, dtype=bool)[0]):
        raise ValueError("row-block 15 must attend exactly t-block 0")
    # coverage: pieces must cover the support exactly once
    cov = np.zeros((16, 16), dtype=np.int32)
    cov[0, :] += 1
    cov[1:16, 0] += 1
    for fb in range(1, 15):
        for tb in ({fb - 1, fb, fb + 1} & set(range(16))) - {0}:
            cov[fb, tb] += 1
        for tb in rand[fb]:
            cov[fb, tb] += 1
    if not np.array_equal(cov, bm.astype(np.int32)):
        raise ValueError("decomposition does not cover the mask exactly")
    return rand


def _exp_dve_ops():
    """Register (once) the two custom DVE ops that evaluate exp on the
    Vector engine: exp(s*x) = core(x)^128 with core = 1 + t + t^2/2,
    t = s*x/128 (s = 0.125 softmax scale).  Log-domain error s^3*x^3/
    (6*128^2) ~ 1.3e-3 at |s*x| = 5 — comparable to fp16 rounding.
    Registration follows the documented per-NEFF DVE-table path
    (bass_utils.dve_table_for_ops resolves names via dve_ops.OPS).
    """
    if "dve_ops" in _CACHE:
        return _CACHE["dve_ops"]
    import numpy as np
    from concourse.dve_spec import Spec, Src0, C0, C1, One, sq, lower
    from concourse.dve_ops import (
        DveOp, OPS, CUSTOM_DVE_SPECS, _SUB_OPCODE_FOR_NAME)
    from concourse.dve_uop import DveOpSpec

    def register(name, spec, rd1):
        if name in _SUB_OPCODE_FOR_NAME:
            return next(op for op in OPS if op.name == name)
        row = max(_SUB_OPCODE_FOR_NAME.values()) + 1
        assert row < 0x20
        shas = {}
        for ver in ("v3", "v4"):
            s = DveOpSpec(name=name, opcode=row, uops=lower(spec, ver=ver),
                          rd1_en=rd1)
            shas[ver] = s.sha(ver)
        op = DveOp(name, spec, subdim=False, uops_sha=shas)
        _SUB_OPCODE_FOR_NAME[name] = row
        CUSTOM_DVE_SPECS[name] = spec
        OPS.append(op)
        return op

    def core_ref(in0, in1, c0, c1, c2):
        x = np.asarray(in0, np.float32)
        t = x * np.float32(c0)
        m = x * np.float32(c1)
        return (np.float32(1.0) + t) + m * m

    def sq7_ref(in0, in1, c0, c1, c2):
        p = np.asarray(in0, np.float32)
        for _ in range(7):
            p = p * p
        return p

    core = register(
        "EXP_CORE_D2_ANT",
        Spec(body=(One + Src0 * C0) + sq(Src0 * C1), reference=core_ref),
        rd1=False)
    x = Src0
    for _ in range(7):
        x = sq(x)
    sq7 = register("EXP_SQ7_ANT", Spec(body=x, reference=sq7_ref), rd1=False)
    _CACHE["dve_ops"] = (core, sq7)
    return core, sq7


# Per-(pair, tile) exp chunking: [c0, c1, engine].  "D" chunks run on the
# Vector engine (2 custom ops via an f32 scratch), offloading ~1/6 of the
# exp columns from the critical Activation engine.  Pair 3 offloads its
# a-piece chunk (tile 0) instead of tile 2 so the DVE never gates the tail;
# pair 0 keeps tile 0 on ACT (head-critical) with a 3-way split so the
# first exp starts after only 4 a-piece matmuls.
_EXP_CHUNKS = {
    (0, 0): [(0, 256, "A"), (256, 512, "A"), (512, 1536, "A")],
    (0, 1): [(0, 512, "D"), (512, 1024, "A")],
    (1, 1): [(0, 512, "D"), (512, 1024, "A")],
    (2, 1): [(0, 512, "D"), (512, 1024, "A")],
    (0, 2): [(0, 512, "D"), (512, 1088, "A")],
    (1, 2): [(0, 512, "D"), (512, 1088, "A")],
    (2, 2): [(0, 512, "D"), (512, 1088, "A")],
    (3, 0): [(0, 512, "D"), (512, 1536, "A")],
    (3, 2): [(0, 512, "A"), (512, 1024, "A"), (1024, 1088, "A")],
}

_EXP_S = 0.125                  # softmax 1/sqrt(d)
_EXP_N = 128.0                  # squaring ladder height (2^7)


def _build_nc():
    """Build + finalize the per-core Bass program (identical on all cores)."""
    import concourse.tile as tile
    from concourse import bacc, mybir

    tiles_qk, tiles_pv, pv_starts = _build_plan()
    exp_core, exp_sq7 = _exp_dve_ops()

    nc = bacc.Bacc(None, target_bir_lowering=False)
    f16 = mybir.dt.float16
    f32 = mybir.dt.float32

    qkt = nc.dram_tensor("qkt", [PAIRS, 64, QKT_W], f16, kind="ExternalInput")
    v1 = nc.dram_tensor("v1", [PAIRS, 128, VC_W], f16, kind="ExternalInput")
    out = nc.dram_tensor("out", [PAIRS, 128, 8, D], f16, kind="ExternalOutput")

    Exp = mybir.ActivationFunctionType.Exp

    with tile.TileContext(nc) as tc:
        with (
            tc.tile_pool(name="io", bufs=4) as io_pool,
            tc.tile_pool(name="pt", bufs=4) as pt_pool,
            tc.tile_pool(name="res", bufs=6) as res_pool,
            tc.tile_pool(name="esc", bufs=2) as esc_pool,
            tc.tile_pool(name="spsum", bufs=2, space="PSUM") as s_psum,
            tc.tile_pool(name="opsum", bufs=2, space="PSUM") as o_psum,
        ):
            state = {}

            # ACT table preload: a 1-col exp on a memset scratch makes the
            # 1.28us activation-table load happen during the DMA head phase
            # instead of on the first real exp.
            scr = res_pool.tile([128, 2], f32, tag="scr")
            scr16 = res_pool.tile([128, 2], f16, tag="scr16")
            nc.vector.memset(scr[:], 0.0)
            nc.scalar.activation(scr16[:, 0:1], scr[:, 0:1], Exp)

            def emit_load(p):
                kq = io_pool.tile([64, QKT_W], f16, tag="kq")
                vc = io_pool.tile([128, VC_W], f16, tag="vc")
                # qkt on the gpsimd queue, vc on sync: transfers parallelize
                # across queues. Pair 0 splits off the tiny slices its first
                # matmuls need (the first-arriving transfer should be minimal)
                krem_mid = KREM_OFF + KREM_SPLIT * BS
                if p == 0:
                    nc.gpsimd.dma_start(out=kq[:, KT_OFF:KT_OFF + 128],
                                        in_=qkt[p, :, KT_OFF:KT_OFF + 128])
                    nc.sync.dma_start(out=kq[:, 0:64], in_=qkt[p, :, 0:64])
                    nc.gpsimd.dma_start(out=kq[:, KT_OFF + 128:KREM_OFF],
                                        in_=qkt[p, :, KT_OFF + 128:KREM_OFF])
                    nc.sync.dma_start(out=kq[:, 64:KT_OFF],
                                      in_=qkt[p, :, 64:KT_OFF])
                    nc.gpsimd.dma_start(out=kq[:, KREM_OFF:krem_mid],
                                        in_=qkt[p, :, KREM_OFF:krem_mid])
                    nc.gpsimd.dma_start(out=kq[:, krem_mid:],
                                        in_=qkt[p, :, krem_mid:])
                elif p in (1, 2):
                    # pairs 1-2: q + dense K ride the sync queue (the gpsimd
                    # queue is busy with the krem streams); krem on gpsimd
                    nc.sync.dma_start(out=kq[:, 0:KREM_OFF],
                                      in_=qkt[p, :, 0:KREM_OFF])
                    nc.gpsimd.dma_start(out=kq[:, KREM_OFF:],
                                        in_=qkt[p, :, KREM_OFF:])
                else:
                    nc.sync.dma_start(out=kq[:, 0:KT_OFF],
                                      in_=qkt[p, :, 0:KT_OFF])
                    nc.gpsimd.dma_start(out=kq[:, KT_OFF:],
                                        in_=qkt[p, :, KT_OFF:])
                nc.sync.dma_start(out=vc[:], in_=v1[p, :, :])
                state[p] = dict(kq=kq, vc=vc, pts=[])

            def emit_front(p, ti):
                st = state[p]
                if ti == 0:
                    # width 512 (not 260): exactly one 2KB PSUM bank per
                    # partition, so the start_tensor_calc zero-region (2KB
                    # granular) aligns exactly with the tile — no marking
                    # bleed into neighbours, no OOB on the last partition
                    o_lo = o_psum.tile([128, 512], f32, tag="o")
                    o_hi = o_psum.tile([128, 512], f32, tag="o")
                    st["o"] = (o_lo, o_hi)
                w = S_TILES[ti]
                s_ps = s_psum.tile([128, w], f32, tag="s")
                kq = st["kq"]
                for pc in tiles_qk[ti]:
                    off = pc["off"]
                    for (p0, p1, kc0, kc1, q0, q1) in pc["qk"]:
                        nc.tensor.matmul(
                            s_ps[p0:p1, off:off + (q1 - q0)],
                            lhsT=kq[:, kc0:kc1], rhs=kq[:, q0:q1],
                            start=True, stop=True,
                        )
                pt = pt_pool.tile([128, w], f16, tag="p")
                st["pts"].append(pt)
                dve_chunks = []
                for (c0, c1, eng) in _EXP_CHUNKS.get((p, ti), [(0, w, "A")]):
                    if eng == "A":
                        nc.scalar.activation(pt[:, c0:c1], s_ps[:, c0:c1],
                                             Exp, scale=_EXP_S)
                    else:
                        dve_chunks.append((s_ps, pt, c0, c1))
                return dve_chunks

            def emit_pv(p, ti):
                st = state[p]
                pt = st["pts"][ti]
                vc = st["vc"]
                for (off, (c0, c1, pp0, pp1, vc0, vc1, vp0, vp1,
                           half, grp, op0, op1)), st_flag in zip(
                               tiles_pv[ti], pv_starts[ti]):
                    o_ps = st["o"][half]
                    nc.tensor.matmul(
                        o_ps[op0:op1, grp * 65:(grp + 1) * 65],
                        lhsT=pt[pp0:pp1, off + c0:off + c1],
                        rhs=vc[vp0:vp1, vc0:vc1],
                        start=st_flag,
                        stop=False,
                        skip_group_check=True,
                    )
                if ti == 1:
                    # o_hi complete after tile 1; normalize it here so the
                    # work overlaps tile 2. Only the last pair stores hi
                    # immediately (other pairs merge into one store below).
                    emit_norm(p, 1)
                    if p == PAIRS - 1:
                        nc.sync.dma_start(out=out[p, :, 4:8, :],
                                          in_=state[p]["os"][:, 4:8, :])
                elif ti == 2:
                    emit_norm(p, 0)
                    os = st["os"]
                    if p == PAIRS - 1:
                        nc.gpsimd.dma_start(out=out[p, :, 0:2, :],
                                            in_=os[:, 0:2, :])
                        nc.sync.dma_start(out=out[p, :, 2:4, :],
                                          in_=os[:, 2:4, :])
                    elif p == 2:
                        nc.sync.dma_start(out=out[p], in_=os[:])
                    else:
                        nc.gpsimd.dma_start(out=out[p], in_=os[:])
                    del state[p]

            def emit_norm(p, half):
                st = state[p]
                if "os" not in st:
                    o_sb = res_pool.tile([128, 8, D], f16, tag="os")
                    recip = res_pool.tile([128, 8], f32, tag="r")
                    st["os"], st["rc"] = o_sb, recip
                o_ps = st["o"][half]
                rc_all = st["rc"][:, half * 4: half * 4 + 4]
                nc.vector.reciprocal(rc_all, o_ps[:, 64:260:65])
                if p == PAIRS - 1 and half == 0:
                    chains = [(0, 2), (2, 4)]
                else:
                    chains = [(0, 4)]
                for (g0, g1) in chains:
                    f0, f1 = half * 4 + g0, half * 4 + g1
                    nc.vector.tensor_mul(
                        st["os"][:, f0:f1, :],
                        o_ps[:, 65 * g0:65 * g1].rearrange(
                            "p (fi c) -> p fi c", c=65)[:, :, 0:64],
                        st["rc"][:, f0:f1].to_broadcast([128, g1 - g0, 64]),
                    )

            for p in range(PAIRS):
                emit_load(p)
            stages = [(p, ti) for p in range(PAIRS) for ti in range(3)]
            n = len(stages)
            pv_next = 0
            def emit_dve_exp(s_ps, pt, c0, c1):
                scr = esc_pool.tile([128, 512], f32, tag="e")
                nc.vector._custom_dve(
                    exp_core, out=scr[:, 0:c1 - c0], in0=s_ps[:, c0:c1],
                    s0=_EXP_S / _EXP_N, s1=_EXP_S / (_EXP_N * 2.0 ** 0.5))
                nc.vector._custom_dve(
                    exp_sq7, out=pt[:, c0:c1], in0=scr[:, 0:c1 - c0])

            for s, (p, ti) in enumerate(stages):
                dve_chunks = emit_front(p, ti)
                # PE prioritizes QK (feeds ACT); drain the lag at the end so
                # the last pair's PV does not pile up after the final exp.
                # Keep lag >= 2 until the last front so pv(3,0) (gated on
                # exp(3,0)) is emitted after QK(3,2), not before it.
                # DVE exp chunks go ahead of this stage's norm muls in
                # the DVE FIFO (the chunks gate PV; stores have slack)
                for ch in dve_chunks:
                    emit_dve_exp(*ch)
                lag = 3 if s < n - 3 else (2 if s < n - 1 else 0)
                while pv_next <= s - lag:
                    emit_pv(*stages[pv_next])
                    pv_next += 1
            while pv_next < n:
                emit_pv(*stages[pv_next])
                pv_next += 1

    nc.finalize()
    return nc


def _prep_inputs(query_layer, key_layer, value_layer, attention_mask):
    """Host-side shard prep: per-core input maps."""
    bf = np.float16
    q = np.asarray(query_layer, dtype=np.float32)
    k = np.asarray(key_layer, dtype=np.float32)
    v = np.asarray(value_layer, dtype=np.float32)
    m = np.asarray(attention_mask, dtype=np.float32)

    # mask must be constant within 64x64 blocks (checked on a few offsets)
    mrow = m[0, :, ::BS, :]                      # [H, NB, T]
    for off in (17, 63):
        if not np.array_equal(mrow, m[0, :, off::BS, :]):
            raise ValueError("mask not constant within 64-row blocks")
    bms = mrow[:, :, ::BS]                       # [H, NB, NB]
    for off in (17, 63):
        if not np.array_equal(bms, mrow[:, :, off::BS]):
            raise ValueError("mask not constant within 64-col blocks")
    bms = bms.astype(bool)

    heads = {h: _head_rand_blocks(bms[h]) for h in range(H)}
    kslots = _krem_slot_map()
    vslots = _vhat_slot_map()

    qT = q.transpose(0, 1, 3, 2).astype(bf)             # [B, H, D, F]
    kT = k.transpose(0, 1, 3, 2).astype(bf)             # [B, H, D, T]
    v1f = np.concatenate(
        [v.astype(bf), np.ones((B, H, T, 1), dtype=bf)], axis=-1
    )                                                   # [B, H, T, 65]
    v1r = v1f.reshape(B, H, 8, 128, 65).transpose(0, 1, 3, 2, 4).reshape(
        B, H, 128, 8 * 65)

    in_maps = []
    pair_index = []
    for c in range(N_CORES):
        qkt = np.zeros((PAIRS, 64, QKT_W), dtype=bf)
        v1c = np.zeros((PAIRS, 128, VC_W), dtype=bf)
        pairs = []
        for p in range(PAIRS):
            h = HEADS_PER_CORE * c + p // B
            b = p % B
            rand = heads[h]

            def blk(src):
                if src[0] == "d":
                    return src[1]
                _, fb, i = src
                r = rand[fb]
                return r[i] if i < len(r) else None

            qkt[p, :, 0:QT_W] = qT[b, h]
            qkt[p, :, KT_OFF:KREM_OFF] = kT[b, h]
            for si, src in enumerate(kslots):
                tb = blk(src)
                if tb is not None:
                    c0 = KREM_OFF + BS * si
                    qkt[p, :, c0:c0 + 64] = kT[b, h][:, BS * tb:BS * tb + 64]
            v1c[p, :, 0:VT_COLS] = v1r[b, h]
            for si, (lo, hi) in enumerate(vslots):
                c0 = VHAT_OFF + 65 * si
                bl, bu = blk(lo), blk(hi)
                if bl is not None:
                    v1c[p, 0:64, c0:c0 + 65] = v1f[b, h, BS * bl:BS * bl + 64]
                if bu is not None:
                    v1c[p, 64:128, c0:c0 + 65] = v1f[b, h, BS * bu:BS * bu + 64]
            pairs.append((b, h))
        in_maps.append({"qkt": qkt, "v1": v1c})
        pair_index.append(pairs)
    return in_maps, pair_index


def kernel(query_layer, key_layer, value_layer, attention_mask):
    from concourse.bass_utils import run_bass_kernel_spmd

    if "nc" not in _CACHE:
        _CACHE["nc"] = _build_nc()
    nc = _CACHE["nc"]

    in_maps, pair_index = _prep_inputs(
        query_layer, key_layer, value_layer, attention_mask
    )
    core_ids = list(range(N_CORES))
    try:
        res = run_bass_kernel_spmd(nc, in_maps, core_ids)
    except Exception:
        # transient device errors clear on redispatch
        res = run_bass_kernel_spmd(nc, in_maps, core_ids)

    out = np.empty((B, F, H, D), dtype=np.float32)
    for c in range(N_CORES):
        core_out = res.results[c]["out"]         # [PAIRS, 128, 8, D]
        for p, (b, h) in enumerate(pair_index[c]):
            # un-permute: f = fo*128 + ti
            out[b, :, h, :] = core_out[p].transpose(1, 0, 2).reshape(F, D)
    return out


# revision 16
# speedup vs baseline: 1.0860x; 1.0556x over previous
"""BigBird block-sparse attention on 8 Trainium2 NeuronCores — sparse exact-cover.

Problem: B=2, H=16, F=T=1024, D=64, 64x64-block BigBird mask per head
(generated at MAX_SEQ_LEN=4096 and cropped to 1024, so there is NO global
last row/col: row-block 15 attends only t-block 0).
  scores = (Q @ K^T) / 8 + (1-mask) * -10000
  out    = softmax(scores) @ V, laid out [B, F, H, D]

Sharding: head-parallel. Core c handles heads {2c, 2c+1} x both batches
= 4 (b,h) pairs; no cross-core communication.

Why sparse: the Activation engine (exp at ~1 col/cycle over 128 lanes)
is the bottleneck. The mask attends only 114 of 256 blocks per head:
  row 0: all 16            col 0: fb 1..15 (15)
  window {fb-1,fb,fb+1} for fb 1..14 (41 after dedup vs col 0)
  3 random blocks per fb 1..14 (42)
We compute S^T = K-slices^T x Q only on attended blocks, packed densely
into PSUM: exp processes exactly 3648 columns/pair (the 114-block floor)
instead of 8192. Skipped blocks contribute exactly 0 — identical to the
reference's exp(score - 1e4) -> 0 underflow in f32 — so no mask bias is
needed and the contraction stays 64.

The PROGRAM is identical on all cores: head-dependence (the random
blocks) is packed by the host into fixed slots of a gathered K region
(krem, appended to [q^T | K^T] in one DRAM tensor per pair) and gathered
V tiles (vhat, appended to the natural V tiles). Window/global pieces
use contiguous K slices and natural V tiles. t-blocks pair into
128-partition pieces; leftover 64-row singles from different from-blocks
share one chunk's two partition halves, so every PSUM column is fully
written (no uninit reads, no wasted exp columns).

S-tile layout per pair: [1536, 1024, 1088].
  tile 0: pieces needing only the dense K^T (a, sh, lcomb) — its QK can
          start as soon as the dense K/Q slices land, minimizing the head.
  tile 1: all remaining o_hi work (krem/vhat pieces, fb >= 8) — o_hi is
          complete after tile 1, so its normalize overlaps tile 2.
  tile 2: o_lo-only (fb < 8).  Its exp window (~0.9us) covers the next
          pair's tile-0 QK, removing the inter-pair ACT bubbles.

PV matmuls lag 3 stages behind the exp front (PE prioritizes QK, which
feeds the critical ACT engine); the lag adaptively shrinks to 0 over the
last stages so the final pair's PV does not pile up after the last exp.
The first exp of pair 0 and the last exp of the last pair are split at
piece boundaries to shorten the pipeline fill/drain.

Softmax denominator: V carries a ones-column (65th); the PV accumulation
yields rowsums for free; one DVE reciprocal + broadcast multiply
normalizes. No max-subtraction needed (scores ~ N(0,1), f32/f16 safe).

Output is stored in the PV-accumulator layout [128, 8, D] (contiguous
1KB DMA runs, no small-element penalty); the host un-permutes
(f = fo*128 + ti).

dtype: fp16 matmul inputs (f32 PSUM accumulate); output stored fp16 on
device (values O(1), fp16 rounding ~5e-4 rel), upcast to f32 on host.
"""

import numpy as np

B, H, F, T, D = 2, 16, 1024, 1024, 64
BS = 64                  # mask block size
NB = F // BS             # 16 blocks per axis
N_CORES = 8
HEADS_PER_CORE = H // N_CORES          # 2
PAIRS = HEADS_PER_CORE * B             # 4 (b,h) pairs per core

# --- static layout constants -------------------------------------------------
# One DRAM tensor per pair: [ q^T (1024) | K^T dense (1024) | krem (43*64) ]
QT_W = 1024
KT_OFF = QT_W
KT_W = 1024                    # dense K^T cols
KREM_OFF = KT_OFF + KT_W       # gathered K blocks appended after dense K^T
N_KREM = 43                    # rand pairs, r3 singles, [K15, r1(14)] + r2,r3(14)
QKT_W = KREM_OFF + N_KREM * BS           # 4800

VT_COLS = 8 * 65               # natural V tiles [t_in 128][tb 8][65]
VHAT_OFF = VT_COLS             # 520
N_VHAT = 31
VC_W = VHAT_OFF + N_VHAT * 65  # 2535

S_TILES = [1536, 1024, 1088]   # PSUM score-tile widths per pair (sum 3648)

_CACHE = {}


def _fb_dest(fb):
    """f-block -> (o_half, col_group, part0, part1) in the PV accumulators."""
    half = 0 if fb < 8 else 1
    grp = (fb % 8) // 2
    p0 = (fb % 2) * 64
    return half, grp, p0, p0 + 64


def _krem(slot, n=1):
    return KREM_OFF + BS * slot, KREM_OFF + BS * (slot + n)


def _kt(c0, c1):
    """Dense K^T cols -> qkt cols."""
    return KT_OFF + c0, KT_OFF + c1


def _vhat(slot):
    return VHAT_OFF + 65 * slot, VHAT_OFF + 65 * (slot + 1)


def _vt(tb, lower):
    """Natural V tile cols for t-block tb; lower half holds even blocks."""
    assert (tb % 2 == 0) == lower
    return 65 * (tb // 2), 65 * (tb // 2) + 65


def _build_plan():
    """Static piece list (identical for every head/core).

    piece: dict(tile, off, w, qk=[...], pv=[...])
      qk op: (p0, p1, kc0, kc1, q0, q1)        out parts p0:p1, free q1-q0
        kc* index the qkt tensor; q* are q cols (qkt cols 0..1024).
      pv op: (c0, c1, pp0, pp1, vc0, vc1, vp0, vp1, half, grp, op0, op1)
        c* cols relative to the piece's s-tile; v* index the v1c tensor.
    """
    pieces = []
    cur_tile, cur_off = 0, 0

    def add(w, qk, pv):
        nonlocal cur_tile, cur_off
        if cur_off + w > S_TILES[cur_tile]:
            assert cur_off == S_TILES[cur_tile], "packing hole"
            cur_tile += 1
            cur_off = 0
        # matmul dests must not cross a PSUM bank (512 f32 cols)
        assert (cur_off % 512) + w <= 512 or w % 512 == 0
        pieces.append(dict(tile=cur_tile, off=cur_off, w=w, qk=qk, pv=pv))
        cur_off += w

    def a_piece(j):
        # fb0 x t-tile j (natural tiles)
        add(64,
            [(0, 128) + _kt(128 * j, 128 * j + 128) + (0, 64)],
            [(0, 64, 0, 128, 65 * j, 65 * j + 65, 0, 128) + _fb_dest(0)])

    def sh(i):
        # t{2i,2i+1} x f{2i,2i+1} shared window piece (natural tiles)
        half, grp, _, _ = _fb_dest(2 * i)
        add(128,
            [(0, 128) + _kt(128 * i, 128 * i + 128) + (128 * i, 128 * i + 128)],
            [(0, 128, 0, 128, 65 * i, 65 * i + 65, 0, 128, half, grp, 0, 128)])

    def lcomb(i):
        # lower=(fb 2i+1, t 2i+2), upper=(fb 2i, t 2i-1); natural parity
        fe, fo = 2 * i, 2 * i + 1
        bl, bu = 2 * i + 2, 2 * i - 1
        add(64,
            [(0, 64) + _kt(64 * bl, 64 * bl + 64) + (64 * fo, 64 * fo + 64),
             (64, 128) + _kt(64 * bu, 64 * bu + 64) + (64 * fe, 64 * fe + 64)],
            [(0, 64, 0, 64) + _vt(bl, True) + (0, 64) + _fb_dest(fo),
             (0, 64, 64, 128) + _vt(bu, False) + (64, 128) + _fb_dest(fe)])

    def w1():
        add(64, [(0, 128) + _kt(64, 192) + (64, 128)],
            [(0, 64, 0, 128) + _vhat(0) + (0, 128) + _fb_dest(1)])

    def w14():
        add(64, [(0, 128) + _kt(832, 960) + (896, 960)],
            [(0, 64, 0, 128) + _vhat(1) + (0, 128) + _fb_dest(14)])
        kc0, kc1 = _krem(18, 2)
        add(64, [(0, 128, kc0, kc1, 896, 960)],
            [(0, 64, 0, 128) + _vhat(2) + (0, 128) + _fb_dest(14)])
        kc0, kc1 = _krem(20, 2)
        add(64, [(0, 128, kc0, kc1, 896, 960)],
            [(0, 64, 0, 128) + _vhat(16) + (0, 128) + _fb_dest(14)])

    def rp(fb):
        kc0, kc1 = _krem(_krem_rp_slot(fb), 2)
        add(64, [(0, 128, kc0, kc1, 64 * fb, 64 * fb + 64)],
            [(0, 64, 0, 128) + _vhat(3 + (fb - 1)) + (0, 128) + _fb_dest(fb)])

    def single(fb):
        # chunk [t0 lower | r3(fb) upper]; both halves share the fb, so one
        # 128-contraction PV op against vhat [V0; Vr3] suffices. QK needs two
        # matmuls (walrus allows only one free dim on the weights AP, so the
        # [K0 | Kr3] gather can't be a single instruction).
        kc0, _ = _krem(_krem_single_slot(fb))
        vc0, vc1 = _vhat(17 + (fb - 1))
        add(64,
            [(0, 64) + _kt(0, 64) + (64 * fb, 64 * fb + 64),
             (64, 128, kc0, kc0 + 64, 64 * fb, 64 * fb + 64)],
            [(0, 64, 0, 128, vc0, vc1, 0, 128) + _fb_dest(fb)])

    def single_1415():
        # fb14/fb15 block-0 singles share one chunk; vhat30 = [V0; V0]
        vc0, vc1 = _vhat(30)
        add(64,
            [(0, 64) + _kt(0, 64) + (896, 960),
             (64, 128) + _kt(0, 64) + (960, 1024)],
            [(0, 64, 0, 64, vc0, vc1, 0, 64) + _fb_dest(14),
             (0, 64, 64, 128, vc0, vc1, 64, 128) + _fb_dest(15)])

    # tile 0 (1536): dense-K-only pieces; a then lcomb (64-wide, fast at the
    # cold PE clock) then sh, so pair 0's exp chunks unlock progressively
    for j in range(8):
        a_piece(j)
    for i in (4, 5, 6, 1):
        lcomb(i)
    for i in range(1, 7):
        sh(i)
    # tile 1 (1024): all remaining o_hi work (krem/vhat pieces)
    w14()
    single_1415()
    for fb in range(8, 14):
        rp(fb)
    for fb in range(8, 14):
        single(fb)
    # tile 2 (1088): o_lo only
    for i in (2, 3):
        lcomb(i)
    for fb in range(1, 8):
        rp(fb)
    w1()
    for fb in range(1, 7):
        single(fb)
    single(7)          # offset 1024: the final 64-col exp chunk

    assert cur_tile == 2 and cur_off == S_TILES[2], (cur_tile, cur_off)
    # all o_hi PV must land by tile 1 (tile 2's PV writes only o_lo, so
    # the hi normalize at tile 1 creates no WAR hazard against them)
    hi_tiles = [pc["tile"] for pc in pieces for op in pc["pv"] if op[8] == 1]
    assert max(hi_tiles) == 1, hi_tiles

    # PV accumulation: the o tiles are memset-zeroed at pair start and every
    # PV matmul is a pure accumulate (start=False). Mixed 64/128-partition
    # accumulation brackets can't be expressed with PSUM zero-region start
    # flags (2KB region granularity x partition range), and a lazy
    # start_tensor_calc would re-mark already-written bytes pending-zero.
    tiles_qk = [[pc for pc in pieces if pc["tile"] == t] for t in range(3)]
    tiles_pv = [[(pc["off"], op) for pc in pieces if pc["tile"] == t
                 for op in pc["pv"]] for t in range(3)]

    # The o accumulators are zeroed by PSUM start_tensor_calc on the first
    # PV op per o-tile instead of DVE memsets: start=True marks the whole
    # 2KB zero region (the o tile's bank) pending-zero; each later op's
    # first touch of a byte replaces instead of accumulating.  The starter
    # must span partitions 0:128, so the full-height sh1 (o_lo) and sh4
    # (o_hi) ops are hoisted to the front of tile-0's PV list.  Every
    # byte of cols 0:260 is eventually written (the exact-cover invariant),
    # so no stale PSUM is ever read.
    pv0 = tiles_pv[0]

    def keyf(e):
        off, op = e
        half, p0, p1 = op[8], op[10], op[11]
        if half == 0 and p0 == 0 and p1 == 128 and off == 768:
            return 0                      # sh1 (o_lo starter)
        if half == 1 and p0 == 0 and p1 == 128 and off == 768 + 3 * 128:
            return 1                      # sh4 (o_hi starter)
        return 2
    pv0.sort(key=keyf)
    assert [keyf(e) for e in pv0[:2]] == [0, 1]
    starts = [[i < 2 for i in range(len(pv0))]] + [
        [False] * len(tiles_pv[t]) for t in (1, 2)]
    return tiles_qk, tiles_pv, starts


# Host packing maps. krem slot -> K source; vhat slot -> (lower64, upper64)
# sources. Source: ('d', tb) dense t-block | ('r', fb, i) i-th rand of fb.
KREM_SPLIT = 22                # tile-1 uses slots [0:22), tile-2 [22:43)


def _krem_rp_slot(fb):
    return 2 * (fb - 8) if fb >= 8 else KREM_SPLIT + 2 * (fb - 1)


def _krem_single_slot(fb):
    return 12 + (fb - 8) if fb >= 8 else KREM_SPLIT + 14 + (fb - 1)


def _krem_slot_map():
    slots = []
    for fb in range(8, 14):
        slots += [("r", fb, 0), ("r", fb, 1)]
    slots += [("r", fb, 2) for fb in range(8, 14)]
    slots += [("d", 15), ("r", 14, 0), ("r", 14, 1), ("r", 14, 2)]
    for fb in range(1, 8):
        slots += [("r", fb, 0), ("r", fb, 1)]
    slots += [("r", fb, 2) for fb in range(1, 8)]
    assert len(slots) == N_KREM
    return slots


def _vhat_slot_map():
    slots = [(("d", 1), ("d", 2)), (("d", 13), ("d", 14)),
             (("d", 15), ("r", 14, 0))]
    slots += [(("r", fb, 0), ("r", fb, 1)) for fb in range(1, 14)]
    slots += [(("r", 14, 1), ("r", 14, 2))]
    slots += [(("d", 0), ("r", fb, 2)) for fb in range(1, 14)]
    slots += [(("d", 0), ("d", 0))]
    assert len(slots) == N_VHAT
    return slots


def _head_rand_blocks(bm):
    """Per-from-block rand lists from a [16,16] block mask, validating the
    cropped-BigBird structure this kernel's decomposition assumes."""
    if not bm[0].all():
        raise ValueError("row-block 0 not global")
    if not bm[:, 0].all():
        raise ValueError("col-block 0 not global")
    rand = {}
    for fb in range(1, 15):
        win = {fb - 1, fb, fb + 1} & set(range(16))
        att = {tb for tb in range(16) if bm[fb, tb]}
        if not win <= att:
            raise ValueError(f"window blocks missing for fb={fb}")
        r = sorted(att - win - {0})
        if len(r) > 3:
            raise ValueError(f"more than 3 rand blocks for fb={fb}")
        rand[fb] = r
    if not np.array_equal(bm[15], np.eye(16, dtype=bool)[0]):
        raise ValueError("row-block 15 must attend exactly t-block 0")
    # coverage: pieces must cover the support exactly once
    cov = np.zeros((16, 16), dtype=np.int32)
    cov[0, :] += 1
    cov[1:16, 0] += 1
    for fb in range(1, 15):
        for tb in ({fb - 1, fb, fb + 1} & set(range(16))) - {0}:
            cov[fb, tb] += 1
        for tb in rand[fb]:
            cov[fb, tb] += 1
    if not np.array_equal(cov, bm.astype(np.int32)):
        raise ValueError("decomposition does not cover the mask exactly")
    return rand


def _exp_dve_ops():
    """Register (once) the two custom DVE ops that evaluate exp on the
    Vector engine: exp(s*x) = core(x)^128 with core = 1 + t + t^2/2,
    t = s*x/128 (s = 0.125 softmax scale).  Log-domain error s^3*x^3/
    (6*128^2) ~ 1.3e-3 at |s*x| = 5 — comparable to fp16 rounding.
    Registration follows the documented per-NEFF DVE-table path
    (bass_utils.dve_table_for_ops resolves names via dve_ops.OPS).
    """
    if "dve_ops" in _CACHE:
        return _CACHE["dve_ops"]
    import numpy as np
    from concourse.dve_spec import Spec, Src0, C0, C1, One, sq, lower
    from concourse.dve_ops import (
        DveOp, OPS, CUSTOM_DVE_SPECS, _SUB_OPCODE_FOR_NAME)
    from concourse.dve_uop import DveOpSpec

    def register(name, spec, rd1):
        if name in _SUB_OPCODE_FOR_NAME:
            return next(op for op in OPS if op.name == name)
        row = max(_SUB_OPCODE_FOR_NAME.values()) + 1
        assert row < 0x20
        shas = {}
        for ver in ("v3", "v4"):
            s = DveOpSpec(name=name, opcode=row, uops=lower(spec, ver=ver),
                          rd1_en=rd1)
            shas[ver] = s.sha(ver)
        op = DveOp(name, spec, subdim=False, uops_sha=shas)
        _SUB_OPCODE_FOR_NAME[name] = row
        CUSTOM_DVE_SPECS[name] = spec
        OPS.append(op)
        return op

    def core_ref(in0, in1, c0, c1, c2):
        x = np.asarray(in0, np.float32)
        t = x * np.float32(c0)
        m = x * np.float32(c1)
        return (np.float32(1.0) + t) + m * m

    def sq7_ref(in0, in1, c0, c1, c2):
        p = np.asarray(in0, np.float32)
        for _ in range(7):
            p = p * p
        return p

    core = register(
        "EXP_CORE_D2_ANT",
        Spec(body=(One + Src0 * C0) + sq(Src0 * C1), reference=core_ref),
        rd1=False)
    x = Src0
    for _ in range(7):
        x = sq(x)
    sq7 = register("EXP_SQ7_ANT", Spec(body=x, reference=sq7_ref), rd1=False)
    _CACHE["dve_ops"] = (core, sq7)
    return core, sq7


# Per-(pair, tile) exp chunking: [c0, c1, engine].  "D" chunks run on the
# Vector engine (2 custom ops via an f32 scratch), offloading ~1/6 of the
# exp columns from the critical Activation engine.  Pair 3 offloads its
# a-piece chunk (tile 0) instead of tile 2 so the DVE never gates the tail;
# pair 0 keeps tile 0 on ACT (head-critical) with a 3-way split so the
# first exp starts after only 4 a-piece matmuls.
_EXP_CHUNKS = {
    (0, 0): [(0, 512, "A"), (512, 768, "A"), (768, 1536, "A")],
    (3, 2): [(0, 512, "A"), (512, 1024, "A"), (1024, 1088, "A")],
}

_EXP_S = 0.125                  # softmax 1/sqrt(d)
_EXP_N = 128.0                  # squaring ladder height (2^7)


def _build_nc():
    """Build + finalize the per-core Bass program (identical on all cores)."""
    import concourse.tile as tile
    from concourse import bacc, mybir

    tiles_qk, tiles_pv, pv_starts = _build_plan()
    exp_core, exp_sq7 = _exp_dve_ops()

    nc = bacc.Bacc(None, target_bir_lowering=False)
    f16 = mybir.dt.float16
    f32 = mybir.dt.float32

    qkt = nc.dram_tensor("qkt", [PAIRS, 64, QKT_W], f16, kind="ExternalInput")
    v1 = nc.dram_tensor("v1", [PAIRS, 128, VC_W], f16, kind="ExternalInput")
    out = nc.dram_tensor("out", [PAIRS, 128, 8, D], f16, kind="ExternalOutput")

    Exp = mybir.ActivationFunctionType.Exp

    with tile.TileContext(nc) as tc:
        with (
            tc.tile_pool(name="io", bufs=4) as io_pool,
            tc.tile_pool(name="pt", bufs=4) as pt_pool,
            tc.tile_pool(name="res", bufs=6) as res_pool,
            tc.tile_pool(name="esc", bufs=2) as esc_pool,
            tc.tile_pool(name="spsum", bufs=2, space="PSUM") as s_psum,
            tc.tile_pool(name="opsum", bufs=2, space="PSUM") as o_psum,
        ):
            state = {}

            # ACT table preload: a 1-col exp on a memset scratch makes the
            # 1.28us activation-table load happen during the DMA head phase
            # instead of on the first real exp.
            scr = res_pool.tile([128, 2], f32, tag="scr")
            scr16 = res_pool.tile([128, 2], f16, tag="scr16")
            nc.vector.memset(scr[:], 0.0)
            nc.scalar.activation(scr16[:, 0:1], scr[:, 0:1], Exp)

            def emit_load(p):
                kq = io_pool.tile([64, QKT_W], f16, tag="kq")
                vc = io_pool.tile([128, VC_W], f16, tag="vc")
                # qkt on the gpsimd queue, vc on sync: transfers parallelize
                # across queues. Pair 0 splits off the tiny slices its first
                # matmuls need (the first-arriving transfer should be minimal)
                krem_mid = KREM_OFF + KREM_SPLIT * BS
                if p == 0:
                    nc.gpsimd.dma_start(out=kq[:, KT_OFF:KT_OFF + 128],
                                        in_=qkt[p, :, KT_OFF:KT_OFF + 128])
                    nc.sync.dma_start(out=kq[:, 0:64], in_=qkt[p, :, 0:64])
                    nc.gpsimd.dma_start(out=kq[:, KT_OFF + 128:KREM_OFF],
                                        in_=qkt[p, :, KT_OFF + 128:KREM_OFF])
                    nc.sync.dma_start(out=kq[:, 64:KT_OFF],
                                      in_=qkt[p, :, 64:KT_OFF])
                    nc.gpsimd.dma_start(out=kq[:, KREM_OFF:krem_mid],
                                        in_=qkt[p, :, KREM_OFF:krem_mid])
                    nc.gpsimd.dma_start(out=kq[:, krem_mid:],
                                        in_=qkt[p, :, krem_mid:])
                elif p in (1, 2):
                    # pairs 1-2: q + dense K ride the sync queue (the gpsimd
                    # queue is busy with the krem streams); krem on gpsimd
                    nc.sync.dma_start(out=kq[:, 0:KREM_OFF],
                                      in_=qkt[p, :, 0:KREM_OFF])
                    nc.gpsimd.dma_start(out=kq[:, KREM_OFF:],
                                        in_=qkt[p, :, KREM_OFF:])
                else:
                    nc.sync.dma_start(out=kq[:, 0:KT_OFF],
                                      in_=qkt[p, :, 0:KT_OFF])
                    nc.gpsimd.dma_start(out=kq[:, KT_OFF:],
                                        in_=qkt[p, :, KT_OFF:])
                nc.sync.dma_start(out=vc[:], in_=v1[p, :, :])
                state[p] = dict(kq=kq, vc=vc, pts=[])

            def emit_front(p, ti):
                st = state[p]
                if ti == 0:
                    # width 512 (not 260): exactly one 2KB PSUM bank per
                    # partition, so the start_tensor_calc zero-region (2KB
                    # granular) aligns exactly with the tile — no marking
                    # bleed into neighbours, no OOB on the last partition
                    o_lo = o_psum.tile([128, 512], f32, tag="o")
                    o_hi = o_psum.tile([128, 512], f32, tag="o")
                    st["o"] = (o_lo, o_hi)
                w = S_TILES[ti]
                s_ps = s_psum.tile([128, w], f32, tag="s")
                kq = st["kq"]
                for pc in tiles_qk[ti]:
                    off = pc["off"]
                    for (p0, p1, kc0, kc1, q0, q1) in pc["qk"]:
                        nc.tensor.matmul(
                            s_ps[p0:p1, off:off + (q1 - q0)],
                            lhsT=kq[:, kc0:kc1], rhs=kq[:, q0:q1],
                            start=True, stop=True,
                        )
                pt = pt_pool.tile([128, w], f16, tag="p")
                st["pts"].append(pt)
                dve_chunks = []
                for (c0, c1, eng) in _EXP_CHUNKS.get((p, ti), [(0, w, "A")]):
                    if eng == "A":
                        nc.scalar.activation(pt[:, c0:c1], s_ps[:, c0:c1],
                                             Exp, scale=_EXP_S)
                    else:
                        dve_chunks.append((s_ps, pt, c0, c1))
                return dve_chunks

            def emit_pv(p, ti):
                st = state[p]
                pt = st["pts"][ti]
                vc = st["vc"]
                for (off, (c0, c1, pp0, pp1, vc0, vc1, vp0, vp1,
                           half, grp, op0, op1)), st_flag in zip(
                               tiles_pv[ti], pv_starts[ti]):
                    o_ps = st["o"][half]
                    nc.tensor.matmul(
                        o_ps[op0:op1, grp * 65:(grp + 1) * 65],
                        lhsT=pt[pp0:pp1, off + c0:off + c1],
                        rhs=vc[vp0:vp1, vc0:vc1],
                        start=st_flag,
                        stop=False,
                        skip_group_check=True,
                    )
                if ti == 1:
                    # o_hi complete after tile 1; normalize it here so the
                    # work overlaps tile 2. Only the last pair stores hi
                    # immediately (other pairs merge into one store below).
                    emit_norm(p, 1)
                    if p == PAIRS - 1:
                        nc.sync.dma_start(out=out[p, :, 4:8, :],
                                          in_=state[p]["os"][:, 4:8, :])
                elif ti == 2:
                    emit_norm(p, 0)
                    os = st["os"]
                    if p == PAIRS - 1:
                        nc.gpsimd.dma_start(out=out[p, :, 0:2, :],
                                            in_=os[:, 0:2, :])
                        nc.sync.dma_start(out=out[p, :, 2:4, :],
                                          in_=os[:, 2:4, :])
                    elif p == 2:
                        nc.sync.dma_start(out=out[p], in_=os[:])
                    else:
                        nc.gpsimd.dma_start(out=out[p], in_=os[:])
                    del state[p]

            def emit_norm(p, half):
                st = state[p]
                if "os" not in st:
                    o_sb = res_pool.tile([128, 8, D], f16, tag="os")
                    recip = res_pool.tile([128, 8], f32, tag="r")
                    st["os"], st["rc"] = o_sb, recip
                o_ps = st["o"][half]
                rc_all = st["rc"][:, half * 4: half * 4 + 4]
                nc.vector.reciprocal(rc_all, o_ps[:, 64:260:65])
                if p == PAIRS - 1 and half == 0:
                    chains = [(0, 2), (2, 4)]
                else:
                    chains = [(0, 4)]
                for (g0, g1) in chains:
                    f0, f1 = half * 4 + g0, half * 4 + g1
                    nc.vector.tensor_mul(
                        st["os"][:, f0:f1, :],
                        o_ps[:, 65 * g0:65 * g1].rearrange(
                            "p (fi c) -> p fi c", c=65)[:, :, 0:64],
                        st["rc"][:, f0:f1].to_broadcast([128, g1 - g0, 64]),
                    )

            for p in range(PAIRS):
                emit_load(p)
            stages = [(p, ti) for p in range(PAIRS) for ti in range(3)]
            n = len(stages)
            pv_next = 0
            def emit_dve_exp(s_ps, pt, c0, c1):
                scr = esc_pool.tile([128, 512], f32, tag="e")
                nc.vector._custom_dve(
                    exp_core, out=scr[:, 0:c1 - c0], in0=s_ps[:, c0:c1],
                    s0=_EXP_S / _EXP_N, s1=_EXP_S / (_EXP_N * 2.0 ** 0.5))
                nc.vector._custom_dve(
                    exp_sq7, out=pt[:, c0:c1], in0=scr[:, 0:c1 - c0])

            for s, (p, ti) in enumerate(stages):
                dve_chunks = emit_front(p, ti)
                # PE prioritizes QK (feeds ACT); drain the lag at the end so
                # the last pair's PV does not pile up after the final exp.
                # Keep lag >= 2 until the last front so pv(3,0) (gated on
                # exp(3,0)) is emitted after QK(3,2), not before it.
                # DVE exp chunks go ahead of this stage's norm muls in
                # the DVE FIFO (the chunks gate PV; stores have slack)
                for ch in dve_chunks:
                    emit_dve_exp(*ch)
                lag = 3 if s < n - 3 else (2 if s < n - 1 else 0)
                while pv_next <= s - lag:
                    emit_pv(*stages[pv_next])
                    pv_next += 1
            while pv_next < n:
                emit_pv(*stages[pv_next])
                pv_next += 1

    nc.finalize()
    return nc


def _prep_inputs(query_layer, key_layer, value_layer, attention_mask):
    """Host-side shard prep: per-core input maps."""
    bf = np.float16
    q = np.asarray(query_layer, dtype=np.float32)
    k = np.asarray(key_layer, dtype=np.float32)
    v = np.asarray(value_layer, dtype=np.float32)
    m = np.asarray(attention_mask, dtype=np.float32)

    # mask must be constant within 64x64 blocks (checked on a few offsets)
    mrow = m[0, :, ::BS, :]                      # [H, NB, T]
    for off in (17, 63):
        if not np.array_equal(mrow, m[0, :, off::BS, :]):
            raise ValueError("mask not constant within 64-row blocks")
    bms = mrow[:, :, ::BS]                       # [H, NB, NB]
    for off in (17, 63):
        if not np.array_equal(bms, mrow[:, :, off::BS]):
            raise ValueError("mask not constant within 64-col blocks")
    bms = bms.astype(bool)

    heads = {h: _head_rand_blocks(bms[h]) for h in range(H)}
    kslots = _krem_slot_map()
    vslots = _vhat_slot_map()

    qT = q.transpose(0, 1, 3, 2).astype(bf)             # [B, H, D, F]
    kT = k.transpose(0, 1, 3, 2).astype(bf)             # [B, H, D, T]
    v1f = np.concatenate(
        [v.astype(bf), np.ones((B, H, T, 1), dtype=bf)], axis=-1
    )                                                   # [B, H, T, 65]
    v1r = v1f.reshape(B, H, 8, 128, 65).transpose(0, 1, 3, 2, 4).reshape(
        B, H, 128, 8 * 65)

    in_maps = []
    pair_index = []
    for c in range(N_CORES):
        qkt = np.zeros((PAIRS, 64, QKT_W), dtype=bf)
        v1c = np.zeros((PAIRS, 128, VC_W), dtype=bf)
        pairs = []
        for p in range(PAIRS):
            h = HEADS_PER_CORE * c + p // B
            b = p % B
            rand = heads[h]

            def blk(src):
                if src[0] == "d":
                    return src[1]
                _, fb, i = src
                r = rand[fb]
                return r[i] if i < len(r) else None

            qkt[p, :, 0:QT_W] = qT[b, h]
            qkt[p, :, KT_OFF:KREM_OFF] = kT[b, h]
            for si, src in enumerate(kslots):
                tb = blk(src)
                if tb is not None:
                    c0 = KREM_OFF + BS * si
                    qkt[p, :, c0:c0 + 64] = kT[b, h][:, BS * tb:BS * tb + 64]
            v1c[p, :, 0:VT_COLS] = v1r[b, h]
            for si, (lo, hi) in enumerate(vslots):
                c0 = VHAT_OFF + 65 * si
                bl, bu = blk(lo), blk(hi)
                if bl is not None:
                    v1c[p, 0:64, c0:c0 + 65] = v1f[b, h, BS * bl:BS * bl + 64]
                if bu is not None:
                    v1c[p, 64:128, c0:c0 + 65] = v1f[b, h, BS * bu:BS * bu + 64]
            pairs.append((b, h))
        in_maps.append({"qkt": qkt, "v1": v1c})
        pair_index.append(pairs)
    return in_maps, pair_index


def kernel(query_layer, key_layer, value_layer, attention_mask):
    from concourse.bass_utils import run_bass_kernel_spmd

    if "nc" not in _CACHE:
        _CACHE["nc"] = _build_nc()
    nc = _CACHE["nc"]

    in_maps, pair_index = _prep_inputs(
        query_layer, key_layer, value_layer, attention_mask
    )
    core_ids = list(range(N_CORES))
    try:
        res = run_bass_kernel_spmd(nc, in_maps, core_ids)
    except Exception:
        # transient device errors clear on redispatch
        res = run_bass_kernel_spmd(nc, in_maps, core_ids)

    out = np.empty((B, F, H, D), dtype=np.float32)
    for c in range(N_CORES):
        core_out = res.results[c]["out"]         # [PAIRS, 128, 8, D]
        for p, (b, h) in enumerate(pair_index[c]):
            # un-permute: f = fo*128 + ti
            out[b, :, h, :] = core_out[p].transpose(1, 0, 2).reshape(F, D)
    return out
